# revision 1
# baseline (speedup 1.0000x reference)
# Trainium2 Bass kernel v2 for the 2-layer R-GCN (see kernel.py for v1).
#
# Key changes vs v1:
#   * Dense edge packing: edges sorted (mg, chunk, window, dstpos, rel) and
#     packed back-to-back into 128-slot gather tiles. Only per-(mg,chunk)
#     call-tail padding remains (~5-12%) vs ~100% group padding in v1.
#   * v-interleaved accumulator: window A-PSUM [128, VW*R] f32 with column
#     enc = dstpos*R + rel. Because tiles are dstpos-sorted, each tile's
#     one-hot S spans only ~60-90 columns -> one short matmul per segment
#     (vs one 128-wide matmul per (window, rel, chunk) group).
#   * S matrices are built SHIFTED (dstenc stores enc - seg_col_start) so one
#     batched tensor_tensor builds S for up to 8 segments at a time.
#   * De-interleave (v,r)->(r,v) happens inside the PSUM->SBUF copy via a
#     permuted access pattern (free).
#   * Transform uses one matmul per relation per megagroup with a strided rhs
#     spanning all windows of the mg.
#   * One shared schedule for both layers (identity slot mapping makes the
#     layer-2 gather table row ids equal to layer-1's).
#
# kernel() takes FULL unsharded inputs and returns the FULL output.

import math
import os

import numpy as np

P = 128          # partitions / edge-tile size
D = 128          # feature dim
R = 8            # relations
VW = 64          # dst window width (A-PSUM = [128, VW*R] f32 = 1 bank)
MGW = 6          # windows per megagroup (6*VW=384 dst rows per mg)
NCORES = 8
NCHUNK = 4
SW = VW * R      # 512: A width
ENC_PAD = 4096.0  # shifted dstenc value that matches no S column
SBUDGET = 768    # max columns per batched S build

_cache = {}


# ----------------------------------------------------------------------------
# Host-side scheduling
# ----------------------------------------------------------------------------

def _schedule(src, dst, etype, n_nodes):
    E = src.shape[0]
    nc_nodes = n_nodes // NCORES
    nwin = math.ceil(nc_nodes / VW)            # 196
    nmg = math.ceil(nwin / MGW)                # 33
    chunk_rows = math.ceil(n_nodes / NCHUNK)   # 25000
    assert chunk_rows <= 32767

    core = dst // nc_nodes
    dl = dst - core * nc_nodes
    w = dl // VW
    v = dl - w * VW
    mg = w // MGW
    c = src // chunk_rows
    local = (src - c * chunk_rows).astype(np.int16)
    enc = (v * R + etype).astype(np.int64)     # 0..SW-1

    # call sizing (shared): per (mg, c), max count over cores, round to 128
    gid = (core * nmg + mg) * NCHUNK + c
    counts = np.bincount(gid, minlength=NCORES * nmg * NCHUNK).reshape(
        NCORES, nmg, NCHUNK
    )
    call_n = (-(-counts.max(axis=0) // P)) * P          # [nmg, NCHUNK]
    call_tiles = call_n // P
    tile_base = np.concatenate([[0], np.cumsum(call_tiles.reshape(-1))[:-1]]
                               ).reshape(nmg, NCHUNK)
    Ttot = int(call_tiles.sum())

    # per-core slot assignment + per-edge tile/partition
    slot = np.zeros(E, dtype=np.int64)
    for cc in range(NCORES):
        es = np.flatnonzero(core == cc)
        key = ((mg[es] * NCHUNK + c[es]) * nwin + w[es]) * SW + enc[es]
        o = np.argsort(key, kind="stable")
        es = es[o]
        g = mg[es] * NCHUNK + c[es]
        gstart = np.searchsorted(g, np.arange(nmg * NCHUNK))
        pos = np.arange(es.shape[0]) - gstart[g]
        slot[es] = tile_base.reshape(-1)[g] * P + pos
    tile = slot // P
    part = slot - tile * P

    # shared segments: per (tile, window) union enc-range over all cores
    segkey = tile * nwin + w
    uniq, inv = np.unique(segkey, return_inverse=True)
    NSEG = uniq.shape[0]
    cs = np.full(NSEG, SW, dtype=np.int64)
    ce = np.zeros(NSEG, dtype=np.int64)
    np.minimum.at(cs, inv, enc)
    np.maximum.at(ce, inv, enc)
    ce += 1
    seg_tile = uniq // nwin
    seg_w = uniq - seg_tile * nwin

    # stop flag: last segment (in emission order = seg id order) per window
    last_for_w = np.zeros(nwin, dtype=np.int64)
    np.maximum.at(last_for_w, seg_w, np.arange(NSEG))
    seg_stop = np.arange(NSEG) == last_for_w[seg_w]

    # combined per-tile S: segment j of a tile occupies S columns
    # [seg_off_j, seg_off_j + width_j); dstenc = enc - cs + seg_off
    widths = ce - cs
    seg_off = np.zeros(NSEG, dtype=np.int64)
    tile_sw = np.zeros(Ttot, dtype=np.int64)
    for s in range(NSEG):
        t = seg_tile[s]
        seg_off[s] = tile_sw[t]
        tile_sw[t] += widths[s]
    SWMAX = int(tile_sw.max())

    # per-core device data (dstenc stores SHIFTED enc: enc - seg cs + off)
    idxw = np.zeros((NCORES, 128, (Ttot * P) // 16), dtype=np.int16)
    dstenc = np.full((NCORES, P, Ttot), ENC_PAD, dtype=np.float32)
    segid = inv
    for cc in range(NCORES):
        es = np.flatnonzero(core == cc)
        flat = np.zeros(Ttot * P, dtype=np.int16)
        flat[slot[es]] = local[es]
        w16 = flat.reshape(-1, 16).T
        idxw[cc] = np.tile(w16, (8, 1))
        dstenc[cc, part[es], tile[es]] = (
            enc[es] - cs[segid[es]] + seg_off[segid[es]]
        ).astype(np.float32)

    tile_c = np.zeros(Ttot, dtype=np.int64)
    for m in range(nmg):
        for ch in range(NCHUNK):
            t0 = tile_base[m, ch]
            tile_c[t0 : t0 + call_tiles[m, ch]] = ch

    return dict(
        nc_nodes=nc_nodes, nwin=nwin, nmg=nmg, chunk_rows=chunk_rows,
        call_n=call_n, call_tiles=call_tiles, tile_base=tile_base, Ttot=Ttot,
        NSEG=NSEG, seg_tile=seg_tile, seg_w=seg_w, seg_cs=cs, seg_ce=ce,
        seg_stop=seg_stop, idxw=idxw, dstenc=dstenc, tile_c=tile_c,
        seg_off=seg_off, tile_sw=tile_sw, SWMAX=SWMAX,
    )


# ----------------------------------------------------------------------------
# Numpy emulator of the device schedule (for schedule debugging)
# ----------------------------------------------------------------------------

def _emulate(sched, feats, W1, loop_w1, b1, W2, loop_w2, b2):
    n_nodes = feats.shape[0]
    nc_nodes = sched["nc_nodes"]
    nwin = sched["nwin"]
    chunk_rows = sched["chunk_rows"]
    Ttot, NSEG = sched["Ttot"], sched["NSEG"]

    def run_layer(table, h_self, W, loop_w, b, relu):
        out = np.zeros((NCORES, nc_nodes, D), dtype=np.float32)
        for cc in range(NCORES):
            dstenc = sched["dstenc"][cc]
            flat = sched["idxw"][cc][:16, :].T.reshape(-1)
            hb = np.zeros((Ttot, P, D), dtype=np.float16)
            for t in range(Ttot):
                ch = sched["tile_c"][t]
                rows = flat[t * P : (t + 1) * P].astype(np.int64)
                hb[t] = table[ch * chunk_rows + rows]
            A = np.zeros((nwin, P, SW), dtype=np.float32)
            for s in range(NSEG):
                t = sched["seg_tile"][s]
                w = sched["seg_w"][s]
                cs, ce = sched["seg_cs"][s], sched["seg_ce"][s]
                off = sched["seg_off"][s]
                iota = np.arange(off, off + ce - cs)
                S = (dstenc[:, t : t + 1] == iota[None, :]).astype(np.float16)
                A[w][:, cs:ce] += (
                    hb[t].astype(np.float32).T @ S.astype(np.float32)
                )
            for w in range(nwin):
                Ar = A[w].reshape(P, VW, R).transpose(0, 2, 1)
                agg = np.zeros((P, VW), dtype=np.float32)
                for r in range(R):
                    agg += W[r].astype(np.float32).T @ Ar[:, r, :]
                v0 = w * VW
                v1 = min(v0 + VW, nc_nodes)
                hT = h_self[cc][v0:v1].astype(np.float32).T
                agg[:, : v1 - v0] += loop_w.astype(np.float32).T @ hT
                o = agg[:, : v1 - v0].T + b[None, :]
                if relu:
                    o = np.maximum(o, 0)
                out[cc, v0:v1] = o
        return out

    table1 = np.zeros((NCHUNK * chunk_rows, D), dtype=np.float16)
    table1[:n_nodes] = feats.astype(np.float16)
    hs = feats.reshape(NCORES, nc_nodes, D).astype(np.float16)
    h1 = run_layer(table1, hs, W1, loop_w1, b1, relu=True)
    h1_16 = h1.astype(np.float16)
    table2 = np.zeros((NCHUNK * chunk_rows, D), dtype=np.float16)
    table2[:n_nodes] = h1_16.reshape(n_nodes, D)
    h2 = run_layer(table2, h1_16, W2, loop_w2, b2, relu=False)
    return h2.reshape(n_nodes, D)


# ----------------------------------------------------------------------------
# Device program
# ----------------------------------------------------------------------------

def _build_program(n_nodes, sched):
    import concourse.bass as bass
    import concourse.mybir as mybir
    import concourse.tile as tile
    from concourse import bacc
    from contextlib import ExitStack

    fp16 = mybir.dt.float16
    f32 = mybir.dt.float32
    i16 = mybir.dt.int16
    AF = mybir.ActivationFunctionType

    nc_nodes = sched["nc_nodes"]
    nwin, nmg = sched["nwin"], sched["nmg"]
    chunk_rows = sched["chunk_rows"]
    call_tiles = sched["call_tiles"]
    tile_base = sched["tile_base"]
    Ttot, NSEG = sched["Ttot"], sched["NSEG"]
    seg_tile = sched["seg_tile"]
    seg_w = sched["seg_w"]
    seg_cs = sched["seg_cs"]
    seg_ce = sched["seg_ce"]
    seg_stop = sched["seg_stop"]
    seg_off = sched["seg_off"]
    tile_sw = sched["tile_sw"]
    SWMAX = sched["SWMAX"]

    segs_of_tile = [[] for _ in range(Ttot)]
    for s in range(NSEG):
        segs_of_tile[seg_tile[s]].append(s)

    nc = bacc.Bacc(
        "TRN2",
        target_bir_lowering=False,
        debug=False,
        enable_asserts=False,
        num_devices=NCORES,
    )

    table1 = nc.dram_tensor(
        "table1", [NCHUNK * chunk_rows, D], fp16, kind="ExternalInput"
    )
    featsT_d = nc.dram_tensor(
        "featsT", [P, nwin * VW], fp16, kind="ExternalInput"
    )
    idx_d = nc.dram_tensor(
        "idxw", [128, (Ttot * P) // 16], i16, kind="ExternalInput"
    )
    enc_d = nc.dram_tensor("dstenc", [P, Ttot], f32, kind="ExternalInput")
    w1_d = nc.dram_tensor("w1e", [P, (R + 1) * D], fp16, kind="ExternalInput")
    w2_d = nc.dram_tensor("w2e", [P, (R + 1) * D], fp16, kind="ExternalInput")
    b1_d = nc.dram_tensor("b1c", [P, 1], f32, kind="ExternalInput")
    b2_d = nc.dram_tensor("b2c", [P, 1], f32, kind="ExternalInput")
    iota_d = nc.dram_tensor("iota512", [P, max(SW, SWMAX)], fp16, kind="ExternalInput")
    zeros_d = nc.dram_tensor("zeros", [P, P], fp16, kind="ExternalInput")
    id16_d = nc.dram_tensor("id16", [P, P], fp16, kind="ExternalInput")
    id32_d = nc.dram_tensor("id32", [P, P], f32, kind="ExternalInput")

    out_d = nc.dram_tensor("out", [nc_nodes, D], f32, kind="ExternalOutput")
    h1shard = nc.dram_tensor("h1shard", [nc_nodes, D], fp16)
    table2 = nc.dram_tensor(
        "table2", [NCHUNK * chunk_rows, D], fp16, addr_space="Shared"
    )

    max_call_tiles = int(call_tiles.max())
    mg_tiles = [int(call_tiles[m].sum()) for m in range(nmg)]
    max_mg_tiles = max(mg_tiles)

    with tile.TileContext(nc) as tc, ExitStack() as ctx:
        consts = ctx.enter_context(tc.tile_pool(name="consts", bufs=1))
        hbp = ctx.enter_context(tc.tile_pool(name="hb", bufs=6))
        sp = ctx.enter_context(tc.tile_pool(name="sbuild", bufs=8))
        asbp = ctx.enter_context(tc.tile_pool(name="asb", bufs=2))
        htp = ctx.enter_context(tc.tile_pool(name="ht", bufs=2))
        rowp = ctx.enter_context(tc.tile_pool(name="rows", bufs=2))
        psA = ctx.enter_context(tc.tile_pool(name="psA", bufs=6, space="PSUM"))
        psG = ctx.enter_context(tc.tile_pool(name="psG", bufs=1, space="PSUM"))

        w1sb = consts.tile([P, (R + 1) * D], fp16, tag="w1")
        w2sb = consts.tile([P, (R + 1) * D], fp16, tag="w2")
        iota = consts.tile([P, max(SW, SWMAX)], fp16, tag="iota")
        zeros = consts.tile([P, P], fp16, tag="zeros")
        id16 = consts.tile([P, P], fp16, tag="id16")
        id32 = consts.tile([P, P], f32, tag="id32")
        b1sb = consts.tile([P, 1], f32, tag="b1")
        b2sb = consts.tile([P, 1], f32, tag="b2")
        encsb = consts.tile([P, Ttot], f32, tag="enc")
        h1T_sb = consts.tile([P, nwin * VW], fp16, tag="h1T")
        featsT_sb = consts.tile([P, nwin * VW], fp16, tag="fT")
        idxsb = consts.tile([128, (Ttot * P) // 16], i16, tag="idx")

        nc.sync.dma_start(out=w1sb[:], in_=w1_d[:])
        nc.sync.dma_start(out=w2sb[:], in_=w2_d[:])
        nc.sync.dma_start(out=iota[:], in_=iota_d[:])
        nc.sync.dma_start(out=zeros[:], in_=zeros_d[:])
        nc.sync.dma_start(out=id16[:], in_=id16_d[:])
        nc.sync.dma_start(out=id32[:], in_=id32_d[:])
        nc.sync.dma_start(out=b1sb[:], in_=b1_d[:])
        nc.sync.dma_start(out=b2sb[:], in_=b2_d[:])
        nc.sync.dma_start(out=encsb[:], in_=enc_d[:])
        nc.sync.dma_start(out=featsT_sb[:], in_=featsT_d[:])
        nc.sync.dma_start(out=idxsb[:], in_=idx_d[:])

        def run_layer(layer, ctx2):
            psT = ctx2.enter_context(
                tc.tile_pool(name=f"psT{layer}", bufs=1, space="PSUM")
            )
            table = table1 if layer == 0 else table2
            wsb = w1sb if layer == 0 else w2sb
            bsb = b1sb if layer == 0 else b2sb
            hT_src = featsT_sb if layer == 0 else h1T_sb

            for m in range(nmg):
                mt = mg_tiles[m]
                mg_t0 = int(tile_base[m, 0])
                w0 = m * MGW
                nw = min(nwin - w0, MGW)

                hb = {}
                for ch in range(NCHUNK):
                    ntc = int(call_tiles[m, ch])
                    if ntc == 0:
                        continue
                    t0 = int(tile_base[m, ch])
                    hbt = hbp.tile([P, max_call_tiles * D], fp16, tag="hb")
                    nc.gpsimd.dma_gather(
                        out_ap=hbt[:, : ntc * D].rearrange(
                            "p (j d) -> p j d", d=D
                        ),
                        in_ap=table[
                            ch * chunk_rows : (ch + 1) * chunk_rows, :
                        ],
                        idxs_ap=idxsb[
                            :, (t0 * P) // 16 : ((t0 + ntc) * P) // 16
                        ],
                        num_idxs=ntc * P,
                        num_idxs_reg=ntc * P,
                        elem_size=D,
                        single_packet=False,
                    )
                    hb[ch] = (hbt, t0)

                Aps = {}
                for wl in range(nw):
                    Apsum = psA.tile([P, SW], f32, tag="A", space="PSUM",
                                     name=f"A{wl}")
                    nc.tensor.matmul(
                        out=Apsum[:], lhsT=zeros[:], rhs=iota[:, :SW],
                        start=True, stop=False,
                    )
                    Aps[w0 + wl] = Apsum

                for ch in range(NCHUNK):
                    if ch not in hb:
                        continue
                    hbt, t0 = hb[ch]
                    for tl in range(int(call_tiles[m, ch])):
                        t = t0 + tl
                        tw = int(tile_sw[t])
                        if tw == 0:
                            continue
                        St = sp.tile([P, SWMAX], fp16, tag="S")
                        nc.vector.tensor_scalar(
                            out=St[:, :tw],
                            in0=iota[:, :tw],
                            scalar1=encsb[:, t : t + 1],
                            scalar2=None,
                            op0=mybir.AluOpType.is_equal,
                        )
                        for s in segs_of_tile[t]:
                            w = int(seg_w[s])
                            cs, ce = int(seg_cs[s]), int(seg_ce[s])
                            wd = ce - cs
                            off = int(seg_off[s])
                            nc.tensor.matmul(
                                out=Aps[w][:, cs:ce],
                                lhsT=hbt[:, tl * D : (tl + 1) * D],
                                rhs=St[:, off : off + wd],
                                start=False,
                                stop=bool(seg_stop[s]),
                            )

                # de-interleave copies (v,r) -> (r,v), f32 -> fp16, into one
                # mg-wide Asb tile [P, nw*SW] (window-major, r-major inside)
                Asb = asbp.tile([P, MGW * SW], fp16, tag="Asb")
                for wl in range(nw):
                    nc.scalar.copy(
                        out=Asb[
                            :, wl * SW : (wl + 1) * SW
                        ].rearrange("p (v r) -> p v r", r=R),
                        in_=Aps[w0 + wl][:].rearrange(
                            "p (v r) -> p v r", r=R
                        ).transpose([0, 2, 1]),
                    )

                # transform: one matmul per relation, strided rhs over windows
                aggP = psG.tile([P, MGW * VW], f32, tag="agg", space="PSUM")
                for r in range(R + 1):
                    if r < R:
                        rhs = Asb[:, : nw * SW].rearrange(
                            "p (w x) -> p w x", x=SW
                        )[:, :, r * VW : (r + 1) * VW]
                    else:
                        rhs = hT_src[
                            :, w0 * VW : (w0 + nw) * VW
                        ].rearrange("p (w x) -> p w x", x=VW)
                    nc.tensor.matmul(
                        out=aggP[:, : nw * VW].rearrange(
                            "p (w x) -> p w x", x=VW
                        ),
                        lhsT=wsb[:, r * D : (r + 1) * D],
                        rhs=rhs,
                        start=(r == 0),
                        stop=(r == R),
                    )

                # epilogue
                r0 = w0 * VW
                nrows = min(nc_nodes - r0, nw * VW)
                ntr = (nw * VW + P - 1) // P
                if layer == 0:
                    nc.scalar.activation(
                        out=h1T_sb[:, r0 : r0 + nw * VW],
                        in_=aggP[:, : nw * VW],
                        func=AF.Relu,
                        bias=bsb[:],
                    )
                    rows_tile = rowp.tile([P, (MGW * VW // P) * D], fp16,
                                          tag="rows16")
                    for j in range(ntr):
                        trp = psT.tile([P, P], fp16, tag="tr", space="PSUM")
                        nc.tensor.transpose(
                            out=trp[:],
                            in_=h1T_sb[:, r0 + j * P : r0 + (j + 1) * P],
                            identity=id16[:],
                        )
                        nc.vector.tensor_copy(
                            out=rows_tile[:, j * D : (j + 1) * D], in_=trp[:]
                        )
                    dst_t = h1shard
                else:
                    oT = htp.tile([P, MGW * VW], f32, tag="oT")
                    nc.scalar.activation(
                        out=oT[:, : nw * VW],
                        in_=aggP[:, : nw * VW],
                        func=AF.Identity,
                        bias=bsb[:],
                    )
                    rows_tile = rowp.tile([P, (MGW * VW // P) * D], f32,
                                          tag="rows32")
                    for j in range(ntr):
                        trp = psT.tile([P, P], f32, tag="tr32", space="PSUM")
                        nc.tensor.transpose(
                            out=trp[:], in_=oT[:, j * P : (j + 1) * P],
                            identity=id32[:],
                        )
                        nc.vector.tensor_copy(
                            out=rows_tile[:, j * D : (j + 1) * D], in_=trp[:]
                        )
                    dst_t = out_d
                full = nrows // P
                if full > 0:
                    nc.sync.dma_start(
                        out=dst_t[r0 : r0 + full * P, :].rearrange(
                            "(j p) d -> p j d", p=P
                        ),
                        in_=rows_tile[:, : full * D].rearrange(
                            "p (j d) -> p j d", d=D
                        ),
                    )
                rem = nrows - full * P
                if rem > 0:
                    nc.sync.dma_start(
                        out=dst_t[r0 + full * P : r0 + nrows, :],
                        in_=rows_tile[:rem, full * D : full * D + D],
                    )

        repeat = int(os.environ.get("KERNEL_REPEAT", "1"))
        for _rep in range(repeat):
            with ExitStack() as c0:
                run_layer(0, c0)
            nc.gpsimd.collective_compute(
                "AllGather",
                mybir.AluOpType.bypass,
                replica_groups=[list(range(NCORES))],
                ins=[h1shard[:]],
                outs=[table2[:n_nodes, :]],
            )
            with ExitStack() as c1:
                run_layer(1, c1)

    nc.compile()
    return nc


# ----------------------------------------------------------------------------
# Entry point
# ----------------------------------------------------------------------------

def _plan(feats, W1, loop_w1, b1, W2, loop_w2, b2, src, dst, etype):
    feats = np.asarray(feats, dtype=np.float32)
    W1 = np.asarray(W1, dtype=np.float32)
    loop_w1 = np.asarray(loop_w1, dtype=np.float32)
    b1 = np.asarray(b1, dtype=np.float32)
    W2 = np.asarray(W2, dtype=np.float32)
    loop_w2 = np.asarray(loop_w2, dtype=np.float32)
    b2 = np.asarray(b2, dtype=np.float32)
    src = np.asarray(src, dtype=np.int64)
    dst = np.asarray(dst, dtype=np.int64)
    etype = np.asarray(etype, dtype=np.int64)

    n_nodes, d = feats.shape
    assert d == D and n_nodes % NCORES == 0 and W1.shape[0] == R

    key = (n_nodes, src.shape[0])
    if key not in _cache:
        sched = _schedule(src, dst, etype, n_nodes)
        prog = _build_program(n_nodes, sched)
        _cache[key] = (sched, prog)
    sched, prog = _cache[key]

    nc_nodes = sched["nc_nodes"]
    nwin = sched["nwin"]
    chunk_rows = sched["chunk_rows"]

    table1 = np.zeros((NCHUNK * chunk_rows, D), dtype=np.float16)
    table1[:n_nodes] = feats.astype(np.float16)

    w1e = np.concatenate([W1, loop_w1[None]], axis=0).astype(np.float16)
    w1e = w1e.transpose(1, 0, 2).reshape(P, (R + 1) * D).copy()
    w2e = np.concatenate([W2, loop_w2[None]], axis=0).astype(np.float16)
    w2e = w2e.transpose(1, 0, 2).reshape(P, (R + 1) * D).copy()
    b1c = np.ascontiguousarray(b1.reshape(P, 1), dtype=np.float32)
    b2c = np.ascontiguousarray(b2.reshape(P, 1), dtype=np.float32)
    iota512 = np.broadcast_to(
        np.arange(max(SW, sched["SWMAX"]), dtype=np.float16),
        (P, max(SW, sched["SWMAX"])),
    ).copy()
    zeros = np.zeros((P, P), dtype=np.float16)
    id16 = np.eye(P, dtype=np.float16)
    id32 = np.eye(P, dtype=np.float32)

    in_maps = []
    for cc in range(NCORES):
        fT = np.zeros((P, nwin * VW), dtype=np.float16)
        fT[:, :nc_nodes] = feats[cc * nc_nodes : (cc + 1) * nc_nodes].astype(
            np.float16
        ).T
        in_maps.append(
            dict(
                table1=table1,
                featsT=fT,
                idxw=sched["idxw"][cc],
                dstenc=sched["dstenc"][cc],
                w1e=w1e,
                w2e=w2e,
                b1c=b1c,
                b2c=b2c,
                iota512=iota512,
                zeros=zeros,
                id16=id16,
                id32=id32,
            )
        )

    def assemble(shards):
        out = np.zeros((n_nodes, D), dtype=np.float32)
        for cc in range(NCORES):
            out[cc * nc_nodes : (cc + 1) * nc_nodes] = shards[cc]
        return out

    return prog, in_maps, assemble


def kernel(feats, W1, loop_w1, b1, W2, loop_w2, b2, src, dst, etype):
    prog, in_maps, assemble = _plan(
        feats, W1, loop_w1, b1, W2, loop_w2, b2, src, dst, etype
    )
    from concourse.bass_utils import run_bass_kernel_spmd

    res = run_bass_kernel_spmd(prog, in_maps, list(range(NCORES)))
    global _last_exec_ns
    _last_exec_ns = res.exec_time_ns

    return assemble([res.results[c]["out"] for c in range(NCORES)])


_last_exec_ns = None

